# revision 13
# baseline (speedup 1.0000x reference)
"""Trainium2 Bass kernel for nn_CorrectSplineLinear (embedding_lookup regime).

Math: reference computes
    W[o,t,f] = sum_c interp[o,t,c] * E[c,f]        (interp = piecewise-linear in t)
    out[o,b,t] = sum_f x[b,f] * W[o,t,f]
which collapses algebraically to
    y[c,b]    = sum_f E[c,f] * x[b,f]              ([128,128] matmul)
    Z[o,s,b]  = sum_c cv[o,s,c] * y[c,b]           ([128,128] matmul per core)
    dZ[o,s,b] = Z[o,s+1,b] - Z[o,s,b]
    out[o,b,t]= Z[o,j(t),b] + tl(t)*dZ[o,j(t),b]
so no [O,I,I] weight is ever materialized.  All device-side I/O is fp16
(the 2e-2 rel-err budget dwarfs fp16's ~7e-4), which halves HBM traffic:
~4.3 MiB of output stores per core at ~350 GB/s.

The expansion (out = Z + tl*dZ, per-partition scalars Z,dZ) is the compute
bottleneck: per-partition-scalar ops force 32 rows x 3 spline segments =
96 tensor_scalar-class ops (measured ~317/521/520 ns per 176-col op on
DVE/ACT/GpSimd).  Rows are padded to 3x176 = 528 columns so every op is
176 wide and stores stay fully contiguous per partition; the host strips
the padding.  Ops are greedily balanced across the three engines and the
store stream is pipelined in row groups behind the expansion.

Front-latency tricks: tl is generated on-device (iota + immediate-scalar
ops) during the input-DMA shadow; inputs arrive as 4 chunked DMAs split
across both HWDGE rings so y matmuls start on first-landed chunks; dZ is
a shifted in-SBUF subtract of Z (no dcv reconstruction on the critical
path).

Sharding: out_features O=256 split across 8 cores (32 rows each); x and E
replicated; each core gets its control_values slice pre-transposed.
"""

import sys
from contextlib import ExitStack

import numpy as np

try:
    import concourse.bass as bass
except ImportError:  # fresh grading dir: concourse lives in the repo checkout
    sys.path.insert(0, "/opt/trn_rl_repo")
    import concourse.bass as bass

import concourse.bacc as bacc
import concourse.mybir as mybir
import concourse.tile as tile
from concourse.bass_utils import run_bass_kernel_spmd

N_CORES = 8
O, I, K, C, B = 256, 512, 3, 128, 128
OL = O // N_CORES  # 32 output rows per core
NS = K + 1  # 4 control values per output row
NZ = OL * NS  # 128 Z columns per core
F16 = mybir.dt.float16
F32 = mybir.dt.float32

# ---- spline geometry (input-independent, mirrors reference arithmetic) ----
_t = np.linspace(0.0, 1.0, I).astype(np.float32)
_ts = (_t * np.float32(K)).astype(np.float32)
_j = np.clip(np.floor(_ts), 0.0, float(K - 1)).astype(np.int32)
_TL = (_ts - _j.astype(np.float32)).astype(np.float32)  # [I] local coord in segment
_b0 = int(np.searchsorted(_j, 1))  # first t index in segment 1 (171)
_b1 = int(np.searchsorted(_j, 2))  # first t index in segment 2 (341)
_SEG = [(0, _b0), (_b0, _b1), (_b1, I)]  # per-segment [t0,t1) in true coords

SW = 172  # padded segment width (even, 4B-aligned); true segments are <= 172
RS = 3 * SW  # padded row stride (528 cols)

# tl over the padded grid is affine in the global padded index i per span:
# tl = (3/511)*i + bias_j for i in [176j, 176j+176)
_TL_SCALE = float(np.float32(3.0) / np.float32(511.0))
_TL_BIAS = [float(_TL_SCALE * (t0 - SW * sj) - sj) for sj, (t0, _t1) in enumerate(_SEG)]

def _padded_tl() -> np.ndarray:
    i = np.arange(RS, dtype=np.float32)
    tlp = np.empty(RS, dtype=np.float32)
    for sj in range(3):
        sl = slice(sj * SW, (sj + 1) * SW)
        tlp[sl] = _TL_SCALE * i[sl] + np.float32(_TL_BIAS[sj])
    return tlp


def _basis128() -> np.ndarray:
    """hat-basis replicated mod 4 across partitions: row p = hat_{p%4} over
    the padded grid; out_row = sum_s Z[o,s]*hat_s reproduces
    Z[o,j] + tl*dZ[o,j] on every span (pad cols are junk, host-stripped).
    The mod-4 periodicity makes every 32-aligned 4-partition slice valid."""
    tlp = _padded_tl()
    bas = np.zeros((4, RS), dtype=np.float32)
    for sj in range(3):
        sl = slice(sj * SW, (sj + 1) * SW)
        bas[sj, sl] = 1.0 - tlp[sl]
        bas[sj + 1, sl] = tlp[sl]
    return np.tile(bas, (32, 1)).astype(np.float16)  # [128, RS]


# ---- packed-input column layout ([128, _TOT] fp16) ----
_CV0 = 0  # cvT [c,(o,s)]: 128 cols
_XE0 = NZ  # 4 chunk-pairs [xT_k | eT_k]: 1024 cols
_BAS0 = _XE0 + 4 * (B + C)  # hat basis (constant): RS cols
_TOT = _BAS0 + RS  # 1668

# PE-path rows: out_row = [Z quad]^T @ hat-basis via 4-partition matmuls.
# PE operands must start at a 32-aligned partition, so ZT2 is computed in
# 4 column-permuted variants placing each PE row's quad at base 0/32/64.
# Variant v'' hosts rows v + 8i (v = 9 + 2*v'', i = 1..3) at base 32*(i-1).
_PE_V = [9, 11, 13, 15]
_PE_ROWS = {v + 8 * i for v in _PE_V for i in range(3)}  # 9,11,..,31 odd

ZSPLIT = 16  # Z columns (4 rows' worth) available early

# store groups (half-open row ranges); the last is small so its HBM
# completion receipt comes quickly after the final expansion op
_STORE_GROUPS = [(0, 1), (1, 2), (2, 4), (4, 7), (7, 10), (10, 14), (14, 18),
                 (18, 22), (22, 25), (25, 28), (28, 30), (30, 31), (31, 32)]

# measured per-op cost (ns) at ~172 cols for greedy engine balancing
_COST = {"v": 255.0, "a": 447.0, "g": 500.0}

_cache: dict = {}


def _schedule_ops():
    """Assign each scalar-path (row, seg) op to DVE or GpSimd; ACT is the
    PSUM drain engine for the PE rows."""
    cost = {"v": _COST["v"], "g": _COST["g"]}
    load = {"v": 900.0, "g": 0.0}  # DVE also drains the ZT2 variants
    plan = []  # (o, seg, engine)
    for o in range(OL):
        if o in _PE_ROWS:
            continue
        for sj in range(3):
            eng = min(load, key=lambda e: load[e] + cost[e])
            load[eng] += cost[eng]
            plan.append((o, sj, eng))
    return plan


def _build_nc():
    nc = bacc.Bacc(
        "TRN2",
        target_bir_lowering=False,
        debug=False,
        num_devices=N_CORES,
        enable_partition_id=False,
        detect_race_conditions=False,
    )
    pk_d = nc.dram_tensor("pk", [128, _TOT], F16, kind="ExternalInput")
    out_d = nc.dram_tensor("out", [B, OL * RS], F16, kind="ExternalOutput")

    with tile.TileContext(nc) as tc, ExitStack() as ctx:
        constp = ctx.enter_context(tc.tile_pool(name="const", bufs=1))
        psump = ctx.enter_context(
            tc.tile_pool(name="psum", bufs=1, space=bass.MemorySpace.PSUM)
        )
        outp = ctx.enter_context(tc.tile_pool(name="outs", bufs=1))

        pk = constp.tile([128, _TOT], F16)
        # chunked input loads split across both HWDGE rings, ordered so the
        # first y-matmul operands and cv land earliest; each chunk's
        # completion receipt (~1.7us) overlaps the later chunks' transfers
        nc.scalar.dma_start(pk[:, 0:384], pk_d[:, 0:384])  # cv + xe0
        nc.sync.dma_start(pk[:, 640:896], pk_d[:, 640:896])  # xe2
        nc.scalar.dma_start(pk[:, 384:640], pk_d[:, 384:640])  # xe1
        nc.sync.dma_start(pk[:, 896:1152], pk_d[:, 896:1152])  # xe3
        nc.scalar.dma_start(pk[:, _BAS0:_TOT], pk_d[:, _BAS0:_TOT])  # basis

        # tl over the padded 528-col grid, generated on-device during the
        # DMA shadow: one fp16 iota (exact for 0..527) + 3 immediate ops
        tlq = constp.tile([128, RS], F16)
        nc.gpsimd.iota(
            tlq[:],
            [[1, RS]],
            base=0,
            channel_multiplier=0,
            allow_small_or_imprecise_dtypes=True,
        )
        for sj in range(3):
            nc.vector.tensor_scalar(
                tlq[:, sj * SW : (sj + 1) * SW],
                tlq[:, sj * SW : (sj + 1) * SW],
                _TL_SCALE,
                _TL_BIAS[sj],
                mybir.AluOpType.mult,
                mybir.AluOpType.add,
            )

        # y[c,b] = sum_f E[c,f] x[b,f]: accumulate over 4 chunks of f,
        # ordered by chunk landing time (scalar ring first, then sync)
        y_ps = psump.tile([128, B], F32)
        for mi, k in enumerate([0, 2, 1, 3]):
            base = _XE0 + k * 256
            nc.tensor.matmul(
                y_ps[:],
                pk[:, base + B : base + B + C],  # lhsT [f_chunk, c]
                pk[:, base : base + B],  # rhs  [f_chunk, b]
                start=(mi == 0),
                stop=(mi == 3),
            )
        y_sb = constp.tile([128, B], F16)
        nc.vector.tensor_copy(y_sb[:], y_ps[:])

        # ZT[b, o*4+s] in two chunks (first 17 cols unblock rows 0-3), then
        # dZT[b, c] = ZT[b, c+1] - ZT[b, c] via shifted in-SBUF subtracts
        # (col 4o+3 of dZT is garbage and never read: j(t) <= 2)
        ztdz = constp.tile([128, 2 * NZ], F32)  # [ZT | dZT]; TS scalars are fp32
        zz_ps1 = psump.tile([128, ZSPLIT + 1], F32)
        zz_ps2 = psump.tile([128, NZ - ZSPLIT - 1], F32)
        zt2_ps = psump.tile([128, 128 * len(_PE_V)], F32)  # permuted ZT2 variants
        zt2 = constp.tile([128, 128 * len(_PE_V)], F16)
        rowp = ctx.enter_context(
            tc.tile_pool(name="rowps", bufs=2, space=bass.MemorySpace.PSUM)
        )

        nc.tensor.matmul(
            zz_ps1[:], y_sb[:], pk[:, _CV0 : _CV0 + ZSPLIT + 1], start=True, stop=True
        )
        nc.vector.tensor_copy(ztdz[:, 0 : ZSPLIT + 1], zz_ps1[:])
        nc.vector.tensor_sub(
            ztdz[:, NZ : NZ + ZSPLIT],
            ztdz[:, 1 : ZSPLIT + 1],
            ztdz[:, 0:ZSPLIT],
        )

        def _ztdz_rest():
            nc.tensor.matmul(
                zz_ps2[:],
                y_sb[:],
                pk[:, _CV0 + ZSPLIT + 1 : _CV0 + NZ],
                start=True,
                stop=True,
            )
            nc.scalar.activation(
                ztdz[:, ZSPLIT + 1 : NZ],
                zz_ps2[:],
                mybir.ActivationFunctionType.Identity,
            )
            nc.vector.tensor_sub(
                ztdz[:, NZ + ZSPLIT : 2 * NZ - 1],
                ztdz[:, ZSPLIT + 1 : NZ],
                ztdz[:, ZSPLIT : NZ - 1],
            )
            # permuted ZT2 variants: variant v'' places the Z quad of row
            # v + 8i at out-partition base 32*(i-1) (PE-aligned)
            cv4 = pk[:, _CV0 : _CV0 + NZ].rearrange(
                "p (i o8 s) -> p i o8 s", i=4, o8=8, s=4
            )
            for vpp, v in enumerate(_PE_V):
                o8 = v - 8
                for i in range(1, 4):
                    nc.tensor.matmul(
                        zt2_ps[32 * (i - 1) : 32 * (i - 1) + 4,
                               128 * vpp : 128 * vpp + 128],
                        cv4[:, i, o8, :],
                        y_sb[:],
                        start=True,
                        stop=True,
                    )
            nc.vector.tensor_copy(zt2[:], zt2_ps[:])

        outs = outp.tile([128, OL * RS], F16)

        plan = _schedule_ops()
        by_row = {}
        for o, sj, eng in plan:
            by_row.setdefault(o, []).append((sj, eng))

        did_rest = False
        for g0, g1 in _STORE_GROUPS:
            if g0 >= ZSPLIT // NS and not did_rest:
                _ztdz_rest()
                did_rest = True
            for o in range(g0, g1):
                col = o * RS
                zc = NS * o
                if o in _PE_ROWS:
                    # PE path: two 4-partition matmuls against the hat
                    # basis; ACT drains PSUM->SBUF fp16 in one op
                    v = [w for w in _PE_V if (o - w) % 8 == 0 and o >= w][0]
                    vpp = _PE_V.index(v)
                    pb = 32 * ((o - v) // 8)
                    row_ps = rowp.tile([128, RS], F32)
                    lhsT = zt2[pb : pb + 4, 128 * vpp : 128 * vpp + 128]
                    nc.tensor.matmul(
                        row_ps[:, 0:512], lhsT,
                        pk[pb : pb + 4, _BAS0 : _BAS0 + 512],
                        start=True, stop=True,
                    )
                    nc.tensor.matmul(
                        row_ps[:, 512:RS], lhsT,
                        pk[pb : pb + 4, _BAS0 + 512 : _BAS0 + RS],
                        start=True, stop=True,
                    )
                    nc.scalar.activation(
                        outs[:, col : col + RS],
                        row_ps[:],
                        mybir.ActivationFunctionType.Identity,
                    )
                    continue
                for sj, eng in by_row[o]:
                    c0 = col + sj * SW
                    s0 = sj * SW
                    if eng == "a":
                        nc.scalar.activation(
                            outs[:, c0 : c0 + SW],
                            tlq[:, s0 : s0 + SW],
                            mybir.ActivationFunctionType.Identity,
                            bias=ztdz[:, zc + sj : zc + sj + 1],
                            scale=ztdz[:, NZ + zc + sj : NZ + zc + sj + 1],
                        )
                    else:
                        veng = nc.vector if eng == "v" else nc.gpsimd
                        veng.tensor_scalar(
                            outs[:, c0 : c0 + SW],
                            tlq[:, s0 : s0 + SW],
                            ztdz[:, NZ + zc + sj : NZ + zc + sj + 1],
                            ztdz[:, zc + sj : zc + sj + 1],
                            mybir.AluOpType.mult,
                            mybir.AluOpType.add,
                        )
            nc.sync.dma_start(
                out_d[:, g0 * RS : g1 * RS], outs[:, g0 * RS : g1 * RS]
            )

    nc.compile()
    return nc


def _get_nc():
    if "nc" not in _cache:
        _cache["nc"] = _build_nc()
    return _cache["nc"]


def _pack_inputs(x, control_values, expansion_matrix):
    x = np.ascontiguousarray(x, dtype=np.float32)
    cv = np.ascontiguousarray(control_values, dtype=np.float32)
    E = np.ascontiguousarray(expansion_matrix, dtype=np.float32)

    base = np.zeros((128, _TOT), dtype=np.float16)
    for k in range(4):
        base[:, _XE0 + k * 256 : _XE0 + k * 256 + B] = x[:, k * 128 : (k + 1) * 128].T
        base[:, _XE0 + k * 256 + B : _XE0 + k * 256 + B + C] = (
            E[:, k * 128 : (k + 1) * 128].T
        )
    base[:, _BAS0:_TOT] = _basis128()

    in_maps = []
    for core in range(N_CORES):
        m = base.copy()
        slab = cv[core * OL : (core + 1) * OL].reshape(OL * NS, C)  # [(o,s), c]
        m[:, _CV0 : _CV0 + NZ] = slab.T
        in_maps.append({"pk": m})
    return in_maps


def _run(in_maps, trace=False):
    nc = _get_nc()
    return run_bass_kernel_spmd(
        nc, in_maps, core_ids=list(range(N_CORES)), trace=trace
    )


def _gather(results):
    # per-core [B, OL*RS] fp16 (padded rows) -> [O, B, I] fp32
    full = np.concatenate(
        [r["out"].reshape(B, OL, 3, SW) for r in results], axis=1
    )  # [B, O, 3, SW]
    out = np.empty((O, B, I), dtype=np.float32)
    fullT = full.transpose(1, 0, 2, 3)  # [O, B, 3, SW]
    for sj, (t0, t1) in enumerate(_SEG):
        out[:, :, t0:t1] = fullT[:, :, sj, 0 : t1 - t0]
    return out


def kernel(x, control_points, control_values, expansion_matrix):
    in_maps = _pack_inputs(x, control_values, expansion_matrix)
    res = _run(in_maps, trace=False)
    return _gather(res.results)


def kernel_traced(x, control_points, control_values, expansion_matrix):
    """Same as kernel() but profiles on HW; returns (out, BassKernelResults)."""
    in_maps = _pack_inputs(x, control_values, expansion_matrix)
    res = _run(in_maps, trace=True)
    return _gather(res.results), res


# revision 14
# speedup vs baseline: 1.1091x; 1.1091x over previous
"""Trainium2 Bass kernel for nn_CorrectSplineLinear (embedding_lookup regime).

Math: reference computes
    W[o,t,f] = sum_c interp[o,t,c] * E[c,f]        (interp = piecewise-linear in t)
    out[o,b,t] = sum_f x[b,f] * W[o,t,f]
which collapses algebraically to
    y[c,b]    = sum_f E[c,f] * x[b,f]              ([128,128] matmul)
    Z[o,s,b]  = sum_c cv[o,s,c] * y[c,b]           ([128,128] matmul per core)
    dZ[o,s,b] = Z[o,s+1,b] - Z[o,s,b]
    out[o,b,t]= Z[o,j(t),b] + tl(t)*dZ[o,j(t),b]
so no [O,I,I] weight is ever materialized.  All device-side I/O is fp16
(the 2e-2 rel-err budget dwarfs fp16's ~7e-4), which halves HBM traffic:
~4.3 MiB of output stores per core at ~350 GB/s.

The expansion (out = Z + tl*dZ, per-partition scalars Z,dZ) is the compute
bottleneck: per-partition-scalar ops force 32 rows x 3 spline segments =
96 tensor_scalar-class ops (measured ~317/521/520 ns per 176-col op on
DVE/ACT/GpSimd).  Rows are padded to 3x176 = 528 columns so every op is
176 wide and stores stay fully contiguous per partition; the host strips
the padding.  Ops are greedily balanced across the three engines and the
store stream is pipelined in row groups behind the expansion.

Front-latency tricks: tl is generated on-device (iota + immediate-scalar
ops) during the input-DMA shadow; inputs arrive as 4 chunked DMAs split
across both HWDGE rings so y matmuls start on first-landed chunks; dZ is
a shifted in-SBUF subtract of Z (no dcv reconstruction on the critical
path).

Sharding: out_features O=256 split across 8 cores (32 rows each); x and E
replicated; each core gets its control_values slice pre-transposed.
"""

import sys
from contextlib import ExitStack

import numpy as np

try:
    import concourse.bass as bass
except ImportError:  # fresh grading dir: concourse lives in the repo checkout
    sys.path.insert(0, "/opt/trn_rl_repo")
    import concourse.bass as bass

import concourse.bacc as bacc
import concourse.mybir as mybir
import concourse.tile as tile
from concourse.bass_utils import run_bass_kernel_spmd

N_CORES = 8
O, I, K, C, B = 256, 512, 3, 128, 128
OL = O // N_CORES  # 32 output rows per core
NS = K + 1  # 4 control values per output row
NZ = OL * NS  # 128 Z columns per core
F16 = mybir.dt.float16
F32 = mybir.dt.float32

# ---- spline geometry (input-independent, mirrors reference arithmetic) ----
_t = np.linspace(0.0, 1.0, I).astype(np.float32)
_ts = (_t * np.float32(K)).astype(np.float32)
_j = np.clip(np.floor(_ts), 0.0, float(K - 1)).astype(np.int32)
_TL = (_ts - _j.astype(np.float32)).astype(np.float32)  # [I] local coord in segment
_b0 = int(np.searchsorted(_j, 1))  # first t index in segment 1 (171)
_b1 = int(np.searchsorted(_j, 2))  # first t index in segment 2 (341)
_SEG = [(0, _b0), (_b0, _b1), (_b1, I)]  # per-segment [t0,t1) in true coords

SW = 172  # padded segment width (even, 4B-aligned); true segments are <= 172
RS = 3 * SW  # padded row stride (528 cols)

# tl over the padded grid is affine in the global padded index i per span:
# tl = (3/511)*i + bias_j for i in [176j, 176j+176)
_TL_SCALE = float(np.float32(3.0) / np.float32(511.0))
_TL_BIAS = [float(_TL_SCALE * (t0 - SW * sj) - sj) for sj, (t0, _t1) in enumerate(_SEG)]

def _padded_tl() -> np.ndarray:
    i = np.arange(RS, dtype=np.float32)
    tlp = np.empty(RS, dtype=np.float32)
    for sj in range(3):
        sl = slice(sj * SW, (sj + 1) * SW)
        tlp[sl] = _TL_SCALE * i[sl] + np.float32(_TL_BIAS[sj])
    return tlp


def _basis128() -> np.ndarray:
    """hat-basis replicated mod 4 across partitions: row p = hat_{p%4} over
    the padded grid; out_row = sum_s Z[o,s]*hat_s reproduces
    Z[o,j] + tl*dZ[o,j] on every span (pad cols are junk, host-stripped).
    The mod-4 periodicity makes every 32-aligned 4-partition slice valid."""
    tlp = _padded_tl()
    bas = np.zeros((4, RS), dtype=np.float32)
    for sj in range(3):
        sl = slice(sj * SW, (sj + 1) * SW)
        bas[sj, sl] = 1.0 - tlp[sl]
        bas[sj + 1, sl] = tlp[sl]
    return np.tile(bas, (32, 1)).astype(np.float16)  # [128, RS]


# ---- packed-input column layout ([128, _TOT] fp16) ----
_CV0 = 0  # cvT [c,(o,s)]: 128 cols
_XE0 = NZ  # 4 chunk-pairs [xT_k | eT_k]: 1024 cols
_BAS0 = _XE0 + 4 * (B + C)  # hat basis (constant): RS cols
_TOT = _BAS0 + RS  # 1668

# PE-path rows: out_row = [Z quad]^T @ hat-basis via 4-partition matmuls.
# PE operands must start at a 32-aligned partition, so ZT2 is computed in
# 4 column-permuted variants placing each PE row's quad at base 0/32/64.
# Variant v'' hosts rows v + 8i (v = 9 + 2*v'', i = 1..3) at base 32*(i-1).
_PE_V = [9, 11, 13, 15]
_PE_ROWS = {v + 8 * i for v in _PE_V for i in range(2)}  # 9,11,..,23 odd

ZSPLIT = 16  # Z columns (4 rows' worth) available early

# store groups (half-open row ranges); the last is small so its HBM
# completion receipt comes quickly after the final expansion op
_STORE_GROUPS = [(0, 1), (1, 2), (2, 4), (4, 7), (7, 10), (10, 14), (14, 18),
                 (18, 22), (22, 25), (25, 28), (28, 30), (30, 31), (31, 32)]

# measured per-op cost (ns) at ~172 cols for greedy engine balancing
_COST = {"v": 255.0, "a": 447.0, "g": 500.0}

_cache: dict = {}


def _schedule_ops():
    """Assign each scalar-path (row, seg) op to DVE or GpSimd; ACT is the
    PSUM drain engine for the PE rows."""
    cost = {"v": _COST["v"], "g": _COST["g"]}
    load = {"v": 900.0, "g": 0.0}  # DVE also drains the ZT2 variants
    plan = []  # (o, seg, engine)
    for o in range(OL):
        if o in _PE_ROWS:
            continue
        for sj in range(3):
            eng = min(load, key=lambda e: load[e] + cost[e])
            load[eng] += cost[eng]
            plan.append((o, sj, eng))
    return plan


def _build_nc():
    nc = bacc.Bacc(
        "TRN2",
        target_bir_lowering=False,
        debug=False,
        num_devices=N_CORES,
        enable_partition_id=False,
        detect_race_conditions=False,
    )
    pk_d = nc.dram_tensor("pk", [128, _TOT], F16, kind="ExternalInput")
    out_d = nc.dram_tensor("out", [B, OL * RS], F16, kind="ExternalOutput")

    with tile.TileContext(nc) as tc, ExitStack() as ctx:
        constp = ctx.enter_context(tc.tile_pool(name="const", bufs=1))
        psump = ctx.enter_context(
            tc.tile_pool(name="psum", bufs=1, space=bass.MemorySpace.PSUM)
        )
        outp = ctx.enter_context(tc.tile_pool(name="outs", bufs=1))

        pk = constp.tile([128, _TOT], F16)
        # chunked input loads split across both HWDGE rings, ordered so the
        # first y-matmul operands and cv land earliest; each chunk's
        # completion receipt (~1.7us) overlaps the later chunks' transfers
        nc.scalar.dma_start(pk[:, 0:384], pk_d[:, 0:384])  # cv + xe0
        nc.sync.dma_start(pk[:, 640:896], pk_d[:, 640:896])  # xe2
        nc.scalar.dma_start(pk[:, 384:640], pk_d[:, 384:640])  # xe1
        nc.sync.dma_start(pk[:, 896:1152], pk_d[:, 896:1152])  # xe3
        nc.scalar.dma_start(pk[:, _BAS0:_TOT], pk_d[:, _BAS0:_TOT])  # basis

        # tl over the padded 528-col grid, generated on-device during the
        # DMA shadow: one fp16 iota (exact for 0..527) + 3 immediate ops
        tlq = constp.tile([128, RS], F16)
        nc.gpsimd.iota(
            tlq[:],
            [[1, RS]],
            base=0,
            channel_multiplier=0,
            allow_small_or_imprecise_dtypes=True,
        )
        for sj in range(3):
            nc.vector.tensor_scalar(
                tlq[:, sj * SW : (sj + 1) * SW],
                tlq[:, sj * SW : (sj + 1) * SW],
                _TL_SCALE,
                _TL_BIAS[sj],
                mybir.AluOpType.mult,
                mybir.AluOpType.add,
            )

        # y[c,b] = sum_f E[c,f] x[b,f]: accumulate over 4 chunks of f,
        # ordered by chunk landing time (scalar ring first, then sync)
        y_ps = psump.tile([128, B], F32)
        for mi, k in enumerate([0, 2, 1, 3]):
            base = _XE0 + k * 256
            nc.tensor.matmul(
                y_ps[:],
                pk[:, base + B : base + B + C],  # lhsT [f_chunk, c]
                pk[:, base : base + B],  # rhs  [f_chunk, b]
                start=(mi == 0),
                stop=(mi == 3),
            )
        y_sb = constp.tile([128, B], F16)
        nc.vector.tensor_copy(y_sb[:], y_ps[:])

        # ZT[b, o*4+s] in two chunks (first 17 cols unblock rows 0-3), then
        # dZT[b, c] = ZT[b, c+1] - ZT[b, c] via shifted in-SBUF subtracts
        # (col 4o+3 of dZT is garbage and never read: j(t) <= 2)
        ztdz = constp.tile([128, 2 * NZ], F32)  # [ZT | dZT]; TS scalars are fp32
        zz_ps1 = psump.tile([128, ZSPLIT + 1], F32)
        zz_ps2 = psump.tile([128, NZ - ZSPLIT - 1], F32)
        zt2_ps = psump.tile([128, 128 * len(_PE_V)], F32)  # permuted ZT2 variants
        zt2 = constp.tile([128, 128 * len(_PE_V)], F16)
        rowp = ctx.enter_context(
            tc.tile_pool(name="rowps", bufs=2, space=bass.MemorySpace.PSUM)
        )

        nc.tensor.matmul(
            zz_ps1[:], y_sb[:], pk[:, _CV0 : _CV0 + ZSPLIT + 1], start=True, stop=True
        )
        nc.vector.tensor_copy(ztdz[:, 0 : ZSPLIT + 1], zz_ps1[:])
        nc.vector.tensor_sub(
            ztdz[:, NZ : NZ + ZSPLIT],
            ztdz[:, 1 : ZSPLIT + 1],
            ztdz[:, 0:ZSPLIT],
        )

        def _ztdz_rest():
            nc.tensor.matmul(
                zz_ps2[:],
                y_sb[:],
                pk[:, _CV0 + ZSPLIT + 1 : _CV0 + NZ],
                start=True,
                stop=True,
            )
            nc.scalar.activation(
                ztdz[:, ZSPLIT + 1 : NZ],
                zz_ps2[:],
                mybir.ActivationFunctionType.Identity,
            )
            nc.vector.tensor_sub(
                ztdz[:, NZ + ZSPLIT : 2 * NZ - 1],
                ztdz[:, ZSPLIT + 1 : NZ],
                ztdz[:, ZSPLIT : NZ - 1],
            )
            # permuted ZT2 variants: variant v'' places the Z quad of row
            # v + 8i at out-partition base 32*(i-1) (PE-aligned)
            cv4 = pk[:, _CV0 : _CV0 + NZ].rearrange(
                "p (i o8 s) -> p i o8 s", i=4, o8=8, s=4
            )
            for vpp, v in enumerate(_PE_V):
                o8 = v - 8
                for i in range(1, 3):
                    nc.tensor.matmul(
                        zt2_ps[32 * (i - 1) : 32 * (i - 1) + 4,
                               128 * vpp : 128 * vpp + 128],
                        cv4[:, i, o8, :],
                        y_sb[:],
                        start=True,
                        stop=True,
                    )
            nc.vector.tensor_copy(zt2[:], zt2_ps[:])

        outs = outp.tile([128, OL * RS], F16)

        plan = _schedule_ops()
        by_row = {}
        for o, sj, eng in plan:
            by_row.setdefault(o, []).append((sj, eng))

        did_rest = False
        for g0, g1 in _STORE_GROUPS:
            if g0 >= ZSPLIT // NS and not did_rest:
                _ztdz_rest()
                did_rest = True
            for o in range(g0, g1):
                col = o * RS
                zc = NS * o
                if o in _PE_ROWS:
                    # PE path: two 4-partition matmuls against the hat
                    # basis; ACT drains PSUM->SBUF fp16 in one op
                    v = [w for w in _PE_V if (o - w) % 8 == 0 and o >= w][0]
                    vpp = _PE_V.index(v)
                    pb = 32 * ((o - v) // 8)
                    row_ps = rowp.tile([128, RS], F32)
                    lhsT = zt2[pb : pb + 4, 128 * vpp : 128 * vpp + 128]
                    nc.tensor.matmul(
                        row_ps[:, 0:512], lhsT,
                        pk[pb : pb + 4, _BAS0 : _BAS0 + 512],
                        start=True, stop=True,
                    )
                    nc.tensor.matmul(
                        row_ps[:, 512:RS], lhsT,
                        pk[pb : pb + 4, _BAS0 + 512 : _BAS0 + RS],
                        start=True, stop=True,
                    )
                    nc.scalar.activation(
                        outs[:, col : col + RS],
                        row_ps[:],
                        mybir.ActivationFunctionType.Identity,
                    )
                    continue
                for sj, eng in by_row[o]:
                    c0 = col + sj * SW
                    s0 = sj * SW
                    if eng == "a":
                        nc.scalar.activation(
                            outs[:, c0 : c0 + SW],
                            tlq[:, s0 : s0 + SW],
                            mybir.ActivationFunctionType.Identity,
                            bias=ztdz[:, zc + sj : zc + sj + 1],
                            scale=ztdz[:, NZ + zc + sj : NZ + zc + sj + 1],
                        )
                    else:
                        veng = nc.vector if eng == "v" else nc.gpsimd
                        veng.tensor_scalar(
                            outs[:, c0 : c0 + SW],
                            tlq[:, s0 : s0 + SW],
                            ztdz[:, NZ + zc + sj : NZ + zc + sj + 1],
                            ztdz[:, zc + sj : zc + sj + 1],
                            mybir.AluOpType.mult,
                            mybir.AluOpType.add,
                        )
            nc.sync.dma_start(
                out_d[:, g0 * RS : g1 * RS], outs[:, g0 * RS : g1 * RS]
            )

    nc.compile()
    return nc


def _get_nc():
    if "nc" not in _cache:
        _cache["nc"] = _build_nc()
    return _cache["nc"]


def _pack_inputs(x, control_values, expansion_matrix):
    x = np.ascontiguousarray(x, dtype=np.float32)
    cv = np.ascontiguousarray(control_values, dtype=np.float32)
    E = np.ascontiguousarray(expansion_matrix, dtype=np.float32)

    base = np.zeros((128, _TOT), dtype=np.float16)
    for k in range(4):
        base[:, _XE0 + k * 256 : _XE0 + k * 256 + B] = x[:, k * 128 : (k + 1) * 128].T
        base[:, _XE0 + k * 256 + B : _XE0 + k * 256 + B + C] = (
            E[:, k * 128 : (k + 1) * 128].T
        )
    base[:, _BAS0:_TOT] = _basis128()

    in_maps = []
    for core in range(N_CORES):
        m = base.copy()
        slab = cv[core * OL : (core + 1) * OL].reshape(OL * NS, C)  # [(o,s), c]
        m[:, _CV0 : _CV0 + NZ] = slab.T
        in_maps.append({"pk": m})
    return in_maps


def _run(in_maps, trace=False):
    nc = _get_nc()
    return run_bass_kernel_spmd(
        nc, in_maps, core_ids=list(range(N_CORES)), trace=trace
    )


def _gather(results):
    # per-core [B, OL*RS] fp16 (padded rows) -> [O, B, I] fp32
    full = np.concatenate(
        [r["out"].reshape(B, OL, 3, SW) for r in results], axis=1
    )  # [B, O, 3, SW]
    out = np.empty((O, B, I), dtype=np.float32)
    fullT = full.transpose(1, 0, 2, 3)  # [O, B, 3, SW]
    for sj, (t0, t1) in enumerate(_SEG):
        out[:, :, t0:t1] = fullT[:, :, sj, 0 : t1 - t0]
    return out


def kernel(x, control_points, control_values, expansion_matrix):
    in_maps = _pack_inputs(x, control_values, expansion_matrix)
    res = _run(in_maps, trace=False)
    return _gather(res.results)


def kernel_traced(x, control_points, control_values, expansion_matrix):
    """Same as kernel() but profiles on HW; returns (out, BassKernelResults)."""
    in_maps = _pack_inputs(x, control_values, expansion_matrix)
    res = _run(in_maps, trace=True)
    return _gather(res.results), res
